# revision 8
# baseline (speedup 1.0000x reference)
# Multi-headed attention (B=8, S=1024, D=1024, H=16) on 8 TRN2 NeuronCores.
# Strategy: pure batch data-parallel (one batch element per core, no
# collectives), all matmuls bf16 with fp32 PSUM accumulation.
#
# Structure (v2, hq-outer): attention runs query-half-outer / pair-inner so
# that the output projection of query blocks 0-3 overlaps the second
# attention half, shrinking the serial O-projection tail. All projection
# work (K pairs 1-7, V chunks, Q halves) is interleaved into the attention
# loops as PE filler so the scalar engine's exp stream starts ~12us in and
# stays covered; the HAM clock gate then never re-throttles the PE.
#
# Key optimizations:
#   - masked key positions are dropped on the host: key/value are gathered to
#     the unmasked positions (padded to a multiple of 128; an exp bias of
#     -30000 zeroes the pads exactly, matching the reference's -1e9 mask).
#     The program is compiled per padded chunk count (nkc) and cached.
#   - weights are pre-banded on the host so every DMA is a plain contiguous
#     [128, N] block transfer, emitted in consumption order.
#   - softmax denominators come free via a ones-column in the V tiles,
#     placed so the denominator lands on PSUM partition 0 where
#     reciprocal_approx_fast reads it in place (no row copy).
#   - dummy warmup matmuls cover the initial DMA window to open the HAM
#     activity window before real work.
#   - the output bias (bv @ Wo + bo) is added on-device during the output
#     projection evacuation; output is written back bf16 to halve the
#     final writeback DMA.
import math
import sys

sys.path.insert(0, "/opt/trn_rl_repo")

from contextlib import ExitStack

import ml_dtypes
import numpy as np

import concourse.bass as bass
import concourse.mybir as mybir
from concourse import bacc
from concourse import tile
from concourse.bass_utils import run_bass_kernel_spmd

dt = mybir.dt
AF = mybir.ActivationFunctionType

B, S, D, H, DK = 8, 1024, 1024, 16, 64
P = 128
NCH = D // P  # 8 chunks of 128 along the 1024-sized dims
NPAIR = H // 2  # 8 head pairs
NEGB = -30000.0  # exp underflows to exactly 0.0, matching the -1e9 masking

_NC_CACHE = {}


def build_nc(nkc: int):
    SK = nkc * P  # gathered+padded key length
    SK2 = SK // 2
    lean = nkc >= 7  # dense-mask fallback: shallower stream buffers
    ET_BUFS = 2 if lean else 4
    OB_BUFS = 2 if lean else 3
    CH_BUFS = 1 if lean else 2
    nc = bacc.Bacc()
    qT = nc.dram_tensor("qT", [D, S], dt.bfloat16, kind="ExternalInput")
    kTg = nc.dram_tensor("kTg", [D, SK], dt.bfloat16, kind="ExternalInput")
    vgb = nc.dram_tensor("vgb", [SK, D], dt.bfloat16, kind="ExternalInput")
    wqb = nc.dram_tensor("wqb", [D, D], dt.bfloat16, kind="ExternalInput")
    wkb = nc.dram_tensor("wkb", [D, D], dt.bfloat16, kind="ExternalInput")
    wv = nc.dram_tensor("wv", [D, D], dt.bfloat16, kind="ExternalInput")
    wo = nc.dram_tensor("wo", [D, D], dt.bfloat16, kind="ExternalInput")
    bq = nc.dram_tensor("bq", [P, NCH], dt.float32, kind="ExternalInput")
    bk = nc.dram_tensor("bk", [P, NCH], dt.float32, kind="ExternalInput")
    msk = nc.dram_tensor("msk", [P, nkc], dt.float32, kind="ExternalInput")
    bo = nc.dram_tensor("bo", [1, D], dt.float32, kind="ExternalInput")
    out = nc.dram_tensor("out", [S, D], dt.bfloat16, kind="ExternalOutput")

    with tile.TileContext(nc) as tc, ExitStack() as ctx:
        big = ctx.enter_context(tc.tile_pool(name="big", bufs=NCH))
        vp = ctx.enter_context(tc.tile_pool(name="vp", bufs=nkc))
        strm = ctx.enter_context(tc.tile_pool(name="strm", bufs=4))
        one = ctx.enter_context(tc.tile_pool(name="one", bufs=1))
        psp = ctx.enter_context(tc.tile_pool(name="psp", bufs=2, space="PSUM"))

        # ---- DMA emission in consumption order -------------------------
        wkb_sb = [None] * NPAIR

        def load_wkb(p):
            t = big.tile([P, D], dt.bfloat16, tag="wkb")
            nc.sync.dma_start(t[:], wkb[p * P : (p + 1) * P, :])
            wkb_sb[p] = t

        load_wkb(0)
        xk = []
        for di in range(NCH):
            t = big.tile([P, SK], dt.bfloat16, tag="xk")
            nc.sync.dma_start(t[:], kTg[di * P : (di + 1) * P, :])
            xk.append(t)

        # PE warmup on a zeroed scratch tile: keeps the HAM activity window
        # busy while the first DMAs land so real work starts at 2.4 GHz.
        scr = one.tile([P, 512], dt.bfloat16, tag="scr")
        nc.gpsimd.memset(scr[:], 0.0)
        wps = psp.tile([P, 512], dt.float32, tag="proj")
        for _ in range(48):
            nc.tensor.matmul(
                wps[:, 0:P], scr[:, 0:P], scr[:, 512 - P : 512], start=True, stop=True
            )

        # small constants
        msk_sb = one.tile([P, nkc], dt.float32, tag="msk")
        nc.sync.dma_start(msk_sb[:], msk[:])
        bq_sb = one.tile([P, NCH], dt.float32, tag="bq")
        nc.sync.dma_start(bq_sb[:], bq[:])
        bk_sb = one.tile([P, NCH], dt.float32, tag="bk")
        nc.sync.dma_start(bk_sb[:], bk[:])
        bo_row = one.tile([1, D], dt.float32, tag="bo_row")
        nc.sync.dma_start(bo_row[:], bo[:])

        # warm the ACT exp table while DMAs stream
        warm = one.tile([1, nkc], dt.float32, tag="warm")
        nc.scalar.activation(warm[:], msk_sb[0:1, :], AF.Exp, bias=0.0, scale=1.0)

        # output-bias row broadcast to all partitions
        bo_sb = one.tile([P, D], dt.float32, tag="bo_sb")
        nc.gpsimd.partition_broadcast(bo_sb[:], bo_row[:])

        # remaining loads, in consumption order
        wqb_sb = [None] * NPAIR

        def load_wqb(p):
            t = big.tile([P, D], dt.bfloat16, tag="wqb")
            nc.sync.dma_start(t[:], wqb[p * P : (p + 1) * P, :])
            wqb_sb[p] = t

        load_wqb(0)
        xq = []
        for di in range(NCH):
            t = big.tile([P, S], dt.bfloat16, tag="xq")
            nc.sync.dma_start(t[:], qT[di * P : (di + 1) * P, :])
            xq.append(t)
        wv_sb = []
        for di in range(NCH):
            t = big.tile([P, D], dt.bfloat16, tag="wv")
            nc.sync.dma_start(t[:], wv[di * P : (di + 1) * P, :])
            wv_sb.append(t)
        vgb_sb = []
        for kc in range(nkc):
            t = vp.tile([P, D], dt.bfloat16, tag="vgb")
            nc.sync.dma_start(t[:], vgb[kc * P : (kc + 1) * P, :])
            vgb_sb.append(t)
        for p in range(1, NPAIR):
            load_wkb(p)
            load_wqb(p)
        wo_sb = []
        for pc in range(NCH):
            t = big.tile([P, D], dt.bfloat16, tag="wo")
            nc.sync.dma_start(t[:], wo[pc * P : (pc + 1) * P, :])
            wo_sb.append(t)

        # ---- work-unit emitters ---------------------------------------
        kt_t = [None] * NPAIR

        def emit_kt_half(p, half):
            # K projection of pair p, key half `half`, [d, s_k] layout
            if kt_t[p] is None:
                kt_t[p] = big.tile([P, SK], dt.bfloat16, tag="kt", name=f"kt{p}")
            hs = slice(half * SK2, (half + 1) * SK2)
            ps = psp.tile([P, 512], dt.float32, tag="proj", name=f"kt_ps{p}_{half}")
            for di in range(NCH):
                nc.tensor.matmul(
                    ps[:, 0:SK2],
                    wkb_sb[p][:, di * P : (di + 1) * P],
                    xk[di][:, hs],
                    start=(di == 0),
                    stop=(di == NCH - 1),
                )
            nc.vector.tensor_scalar_add(
                kt_t[p][:, hs], ps[:, 0:SK2], bk_sb[:, p : p + 1]
            )

        # V tiles: per head a 128-wide group [ones | 63 dead | 64 dims] so
        # the PV output puts the softmax denominator on PSUM partition 0
        # (read in place by reciprocal_approx_fast) and the dims at
        # partitions 64..127 (PSUM partition ranges cannot straddle the 64
        # line). Extra M is free: matmul cost is set by the moving free dim
        # N, not M.
        VW = 128
        vv_t = [None] * nkc

        def emit_vv(kc):
            t = vp.tile([P, H * VW], dt.bfloat16, tag="vv", name=f"vv{kc}")
            vv_t[kc] = t
            nc.gpsimd.memset(t[:], 1.0)
            for half in range(2):
                hs = slice(half * 512, (half + 1) * 512)
                ps = psp.tile([P, 512], dt.float32, tag="proj", name=f"v_ps{kc}_{half}")
                for di in range(NCH):
                    nc.tensor.matmul(
                        ps[:],
                        vgb_sb[kc][:, di * P : (di + 1) * P],
                        wv_sb[di][:, hs],
                        start=(di == 0),
                        stop=(di == NCH - 1),
                    )
                dst = t[:, half * 8 * VW : (half + 1) * 8 * VW].rearrange(
                    "p (h e) -> p h e", e=VW
                )[:, :, 64 : 64 + DK]
                srcv = ps[:].rearrange("p (h e) -> p h e", e=DK)
                nc.vector.tensor_copy(dst, srcv)

        qt_t = [None] * NPAIR

        def emit_qt_half(p, hq):
            if qt_t[p] is None:
                qt_t[p] = big.tile([P, S], dt.bfloat16, tag="qt", name=f"qt{p}")
            qs = slice(hq * 512, (hq + 1) * 512)
            ps = psp.tile([P, 512], dt.float32, tag="proj", name=f"qt_ps{p}_{hq}")
            for di in range(NCH):
                nc.tensor.matmul(
                    ps[:],
                    wqb_sb[p][:, di * P : (di + 1) * P],
                    xq[di][:, qs],
                    start=(di == 0),
                    stop=(di == NCH - 1),
                )
            nc.vector.tensor_scalar_add(qt_t[p][:, qs], ps[:], bq_sb[:, p : p + 1])

        ct_t = [None] * NPAIR

        def emit_oproj(qc, half):
            # output projection for query block qc, D-half `half` (+ bias)
            hs = slice(half * 512, (half + 1) * 512)
            ps = psp.tile([P, 512], dt.float32, tag="proj", name=f"o_ps{qc}_{half}")
            for pc in range(NCH):
                nc.tensor.matmul(
                    ps[:],
                    ct_t[pc][:, qc * P : (qc + 1) * P],
                    wo_sb[pc][:, hs],
                    start=(pc == 0),
                    stop=(pc == NCH - 1),
                )
            ob = strm.tile([P, 512], dt.bfloat16, tag="ob", bufs=OB_BUFS)
            nc.vector.tensor_add(ob[:], ps[:], bo_sb[:, hs])
            nc.sync.dma_start(out[qc * P : (qc + 1) * P, hs], ob[:])

        def emit_dummies(n, name):
            dps = psp.tile([P, 512], dt.float32, tag="proj", name=name)
            for _ in range(n):
                nc.tensor.matmul(
                    dps[:, 0:P],
                    scr[:, 0:P],
                    scr[:, 512 - P : 512],
                    start=True,
                    stop=True,
                )

        # ---- filler schedule ------------------------------------------
        # phase 1 (hq=0): pair p slots feed K/Q projections of pair p+1;
        # pair 7 slots start Q-half-1 projections for phase 2.
        # phase 2 (hq=1): slots drain a queue of remaining Q-half-1
        # projections and the O-projection of query blocks 0-3 (whose ct
        # rows completed in phase 1).
        p2_queue = []
        for i in range(6):
            p2_queue.append(("q1", i + 2))
            p2_queue.append(("o", i // 2, i % 2))
        p2_queue.append(("o", 3, 0))
        p2_queue.append(("o", 3, 1))
        oproj_done = set((u[1], u[2]) for u in p2_queue if u[0] == "o")

        def filler(hq, p, kc):
            if hq == 0:
                if p == NPAIR - 1:
                    if kc == 1:
                        emit_qt_half(0, 1)
                    elif kc == 2:
                        emit_qt_half(1, 1)
                elif kc == 1:
                    emit_kt_half(p + 1, 0)
                elif kc == 2:
                    emit_kt_half(p + 1, 1)
                elif kc == 3:
                    emit_qt_half(p + 1, 0)
            else:
                if kc in (1, 3) and p2_queue:
                    u = p2_queue.pop(0)
                    if u[0] == "q1":
                        emit_qt_half(u[1], 1)
                    else:
                        emit_oproj(u[1], u[2])
                elif kc == 2 and p == NPAIR - 1:
                    # no projection work left: keep the clock gate open
                    emit_dummies(4, f"d{p}_{kc}")

        # ---- pre-phase: K/Q projections of pair 0 ----------------------
        emit_kt_half(0, 0)
        emit_kt_half(0, 1)
        emit_qt_half(0, 0)

        # ---- attention: hq outer, pair inner ---------------------------
        for hq in range(2):
            for p in range(NPAIR):
                if ct_t[p] is None:
                    ct_t[p] = big.tile([P, S], dt.bfloat16, tag="ct", name=f"ct{p}")
                ct = ct_t[p]
                qs = slice(hq * 512, (hq + 1) * 512)
                pv0 = psp.tile([P, 512], dt.float32, tag="pv")
                pv1 = psp.tile([P, 512], dt.float32, tag="pv")
                for kc in range(nkc):
                    st = psp.tile([P, 1024], dt.float32, tag="st")
                    ks = slice(kc * P, (kc + 1) * P)
                    nc.tensor.matmul(
                        st[:, 0:512],
                        kt_t[p][0:DK, ks],
                        qt_t[p][0:DK, qs],
                        start=True,
                        stop=True,
                        tile_position=(0, 0),
                    )
                    nc.tensor.matmul(
                        st[:, 512:1024],
                        kt_t[p][DK:P, ks],
                        qt_t[p][DK:P, qs],
                        start=True,
                        stop=True,
                        tile_position=(DK, 0),
                    )
                    et = strm.tile([P, 1024], dt.bfloat16, tag="et", bufs=ET_BUFS)
                    nc.scalar.activation(
                        et[:], st[:], AF.Exp, bias=msk_sb[:, kc : kc + 1], scale=1.0
                    )
                    if hq == 0 and p == 0:
                        # JIT V projection: chunk kc right before its first use
                        emit_vv(kc)
                    else:
                        filler(hq, p, kc)
                    nc.tensor.matmul(
                        pv0[:],
                        vv_t[kc][:, (2 * p) * VW : (2 * p + 1) * VW],
                        et[:, 0:512],
                        start=(kc == 0),
                        stop=(kc == nkc - 1),
                    )
                    nc.tensor.matmul(
                        pv1[:],
                        vv_t[kc][:, (2 * p + 1) * VW : (2 * p + 2) * VW],
                        et[:, 512:1024],
                        start=(kc == 0),
                        stop=(kc == nkc - 1),
                    )
                if hq == 0 and p == 0:
                    # pair 0 used its slots for V; emit pair 1's K/Q here
                    emit_kt_half(1, 0)
                    emit_kt_half(1, 1)
                    emit_qt_half(1, 0)
                for hloc, pv in ((0, pv0), (1, pv1)):
                    # pv partition 0 holds the softmax denominator; the dims
                    # sit at partitions 32..95 (32-aligned PSUM access)
                    rcp = strm.tile([1, 512], dt.float32, tag="rcp", bufs=CH_BUFS)
                    nc.vector.reciprocal_approx_fast(rcp[:], pv[0:1, :])
                    rb = strm.tile([DK, 512], dt.float32, tag="rb", bufs=CH_BUFS)
                    nc.gpsimd.partition_broadcast(rb[:], rcp[:])
                    nc.vector.tensor_mul(
                        ct[hloc * DK : (hloc + 1) * DK, qs], pv[64 : 64 + DK, :], rb[:]
                    )

        # ---- output projection tail (query blocks not drained early) ---
        for qc in range(NCH):
            for half in range(2):
                if (qc, half) not in oproj_done:
                    emit_oproj(qc, half)

    nc.finalize()
    return nc


def _band(w: np.ndarray, ncol: int) -> np.ndarray:
    # w: [1024, ncol*128]. Output row-block p holds column-band p rearranged
    # as [128 rows (r), 8 chunks (di) x 128]: out[p*128+r, di*128+c] =
    # w[di*128+r, p*128+c]  -- the stationary layout for lhsT slices.
    return np.ascontiguousarray(
        w.reshape(NCH, P, ncol, P).transpose(2, 1, 0, 3).reshape(ncol * P, D)
    )


def _make_in_maps(query, key, value, mask, Wq, bq, Wk, bk, Wv, bv, Wo, bo):
    query = np.asarray(query, dtype=np.float32)
    key = np.asarray(key, dtype=np.float32)
    value = np.asarray(value, dtype=np.float32)
    mask = np.asarray(mask)
    Wq = np.asarray(Wq, dtype=np.float32)
    Wk = np.asarray(Wk, dtype=np.float32)
    Wv = np.asarray(Wv, dtype=np.float32)
    Wo = np.asarray(Wo, dtype=np.float32)
    sc = np.float32(1.0 / math.sqrt(DK))
    bo_eff = (np.asarray(bv, np.float32) @ Wo + np.asarray(bo, np.float32)).reshape(
        1, D
    )

    idxs, nv = [], []
    for i in range(B):
        ix = np.nonzero(np.asarray(mask[i, 0]) != 0)[0]
        idxs.append(ix)
        nv.append(len(ix))
    nkc = min(NCH, max(1, -(-max(nv) // P)))
    SK = nkc * P

    bf16 = ml_dtypes.bfloat16
    wqb = _band(Wq * sc, NCH).astype(bf16)
    wkb = _band(Wk, NCH).astype(bf16)
    wv_b = np.ascontiguousarray(Wv).astype(bf16)
    wo_b = np.ascontiguousarray(Wo).astype(bf16)
    bq2 = np.ascontiguousarray((np.asarray(bq, np.float32) * sc).reshape(NCH, P).T)
    bk2 = np.ascontiguousarray(np.asarray(bk, np.float32).reshape(NCH, P).T)

    in_maps = []
    for i in range(B):
        ix = idxs[i]
        pad = SK - len(ix)
        ixp = np.concatenate([ix, np.zeros(pad, dtype=ix.dtype)])
        mb = np.full(SK, 0.0, dtype=np.float32)
        if pad:
            mb[len(ix) :] = NEGB
        kTg = np.ascontiguousarray(key[i][ixp].astype(bf16).T)
        vT = value[i][ixp].astype(bf16).T  # [D, SK]
        vgb = np.ascontiguousarray(_band(vT, nkc))
        in_maps.append(
            {
                "qT": np.ascontiguousarray(query[i].astype(bf16).T),
                "kTg": kTg,
                "vgb": vgb,
                "wqb": wqb,
                "wkb": wkb,
                "wv": wv_b,
                "wo": wo_b,
                "bq": bq2,
                "bk": bk2,
                "msk": np.ascontiguousarray(mb.reshape(nkc, P).T),
                "bo": bo_eff,
            }
        )
    return nkc, in_maps


def kernel(query, key, value, mask, Wq, bq, Wk, bk, Wv, bv, Wo, bo):
    nkc, in_maps = _make_in_maps(
        query, key, value, mask, Wq, bq, Wk, bk, Wv, bv, Wo, bo
    )
    if nkc not in _NC_CACHE:
        _NC_CACHE[nkc] = build_nc(nkc)
    nc = _NC_CACHE[nkc]
    res = run_bass_kernel_spmd(nc, in_maps, list(range(B)))
    return np.stack([res.results[i]["out"] for i in range(B)], axis=0).astype(
        np.float32
    )


# revision 9
# speedup vs baseline: 1.0164x; 1.0164x over previous
# Multi-headed attention (B=8, S=1024, D=1024, H=16) on 8 TRN2 NeuronCores.
# Strategy: pure batch data-parallel (one batch element per core, no
# collectives), all matmuls bf16 with fp32 PSUM accumulation.
#
# Structure (v2, hq-outer): attention runs query-half-outer / pair-inner so
# that the output projection of query blocks 0-3 overlaps the second
# attention half, shrinking the serial O-projection tail. All projection
# work (K pairs 1-7, V chunks, Q halves) is interleaved into the attention
# loops as PE filler so the scalar engine's exp stream starts ~12us in and
# stays covered; the HAM clock gate then never re-throttles the PE.
#
# Key optimizations:
#   - masked key positions are dropped on the host: key/value are gathered to
#     the unmasked positions (padded to a multiple of 128; an exp bias of
#     -30000 zeroes the pads exactly, matching the reference's -1e9 mask).
#     The program is compiled per padded chunk count (nkc) and cached.
#   - weights are pre-banded on the host so every DMA is a plain contiguous
#     [128, N] block transfer, emitted in consumption order.
#   - softmax denominators come free via a ones-column in the V tiles,
#     placed so the denominator lands on PSUM partition 0 where
#     reciprocal_approx_fast reads it in place (no row copy).
#   - dummy warmup matmuls cover the initial DMA window to open the HAM
#     activity window before real work.
#   - the output bias (bv @ Wo + bo) is added on-device during the output
#     projection evacuation; output is written back bf16 to halve the
#     final writeback DMA.
import math
import sys

sys.path.insert(0, "/opt/trn_rl_repo")

from contextlib import ExitStack

import ml_dtypes
import numpy as np

import concourse.bass as bass
import concourse.mybir as mybir
from concourse import bacc
from concourse import tile
from concourse.bass_utils import run_bass_kernel_spmd

dt = mybir.dt
AF = mybir.ActivationFunctionType

B, S, D, H, DK = 8, 1024, 1024, 16, 64
P = 128
NCH = D // P  # 8 chunks of 128 along the 1024-sized dims
NPAIR = H // 2  # 8 head pairs
NEGB = -30000.0  # exp underflows to exactly 0.0, matching the -1e9 masking

_NC_CACHE = {}


def build_nc(nkc: int):
    SK = nkc * P  # gathered+padded key length
    SK2 = SK // 2
    lean = nkc >= 7  # dense-mask fallback: shallower stream buffers
    ET_BUFS = 2 if lean else 4
    OB_BUFS = 2 if lean else 3
    CH_BUFS = 1 if lean else 2
    nc = bacc.Bacc()
    qT = nc.dram_tensor("qT", [D, S], dt.bfloat16, kind="ExternalInput")
    kTg = nc.dram_tensor("kTg", [D, SK], dt.bfloat16, kind="ExternalInput")
    vgb = nc.dram_tensor("vgb", [SK, D], dt.bfloat16, kind="ExternalInput")
    wqb = nc.dram_tensor("wqb", [D, D], dt.bfloat16, kind="ExternalInput")
    wkb = nc.dram_tensor("wkb", [D, D], dt.bfloat16, kind="ExternalInput")
    wv = nc.dram_tensor("wv", [D, D], dt.bfloat16, kind="ExternalInput")
    wo = nc.dram_tensor("wo", [D, D], dt.bfloat16, kind="ExternalInput")
    bq = nc.dram_tensor("bq", [P, NCH], dt.float32, kind="ExternalInput")
    bk = nc.dram_tensor("bk", [P, NCH], dt.float32, kind="ExternalInput")
    msk = nc.dram_tensor("msk", [P, nkc], dt.float32, kind="ExternalInput")
    bo = nc.dram_tensor("bo", [1, D], dt.float32, kind="ExternalInput")
    out = nc.dram_tensor("out", [S, D], dt.bfloat16, kind="ExternalOutput")

    with tile.TileContext(nc) as tc, ExitStack() as ctx:
        big = ctx.enter_context(tc.tile_pool(name="big", bufs=NCH))
        vp = ctx.enter_context(tc.tile_pool(name="vp", bufs=nkc))
        strm = ctx.enter_context(tc.tile_pool(name="strm", bufs=4))
        one = ctx.enter_context(tc.tile_pool(name="one", bufs=1))
        psp = ctx.enter_context(tc.tile_pool(name="psp", bufs=2, space="PSUM"))

        # ---- DMA emission in consumption order -------------------------
        wkb_sb = [None] * NPAIR

        def load_wkb(p):
            t = big.tile([P, D], dt.bfloat16, tag="wkb")
            nc.sync.dma_start(t[:], wkb[p * P : (p + 1) * P, :])
            wkb_sb[p] = t

        load_wkb(0)
        xk = []
        for di in range(NCH):
            t = big.tile([P, SK], dt.bfloat16, tag="xk")
            nc.sync.dma_start(t[:], kTg[di * P : (di + 1) * P, :])
            xk.append(t)
        load_wkb(1)

        # PE warmup on a zeroed scratch tile: keeps the HAM activity window
        # busy while the first DMAs land so real work starts at 2.4 GHz.
        scr = one.tile([P, 512], dt.bfloat16, tag="scr")
        nc.gpsimd.memset(scr[:], 0.0)
        wps = psp.tile([P, 512], dt.float32, tag="proj")
        for _ in range(48):
            nc.tensor.matmul(
                wps[:, 0:P], scr[:, 0:P], scr[:, 512 - P : 512], start=True, stop=True
            )

        # small constants
        msk_sb = one.tile([P, nkc], dt.float32, tag="msk")
        nc.sync.dma_start(msk_sb[:], msk[:])
        bq_sb = one.tile([P, NCH], dt.float32, tag="bq")
        nc.sync.dma_start(bq_sb[:], bq[:])
        bk_sb = one.tile([P, NCH], dt.float32, tag="bk")
        nc.sync.dma_start(bk_sb[:], bk[:])
        bo_row = one.tile([1, D], dt.float32, tag="bo_row")
        nc.sync.dma_start(bo_row[:], bo[:])

        # warm the ACT exp table while DMAs stream
        warm = one.tile([1, nkc], dt.float32, tag="warm")
        nc.scalar.activation(warm[:], msk_sb[0:1, :], AF.Exp, bias=0.0, scale=1.0)

        # output-bias row broadcast to all partitions
        bo_sb = one.tile([P, D], dt.float32, tag="bo_sb")
        nc.gpsimd.partition_broadcast(bo_sb[:], bo_row[:])

        # remaining loads, in consumption order
        wqb_sb = [None] * NPAIR

        def load_wqb(p):
            t = big.tile([P, D], dt.bfloat16, tag="wqb")
            nc.sync.dma_start(t[:], wqb[p * P : (p + 1) * P, :])
            wqb_sb[p] = t

        load_wqb(0)
        xq = []
        for di in range(NCH):
            t = big.tile([P, S], dt.bfloat16, tag="xq")
            nc.sync.dma_start(t[:], qT[di * P : (di + 1) * P, :])
            xq.append(t)
        wv_sb = []
        for di in range(NCH):
            t = big.tile([P, D], dt.bfloat16, tag="wv")
            nc.sync.dma_start(t[:], wv[di * P : (di + 1) * P, :])
            wv_sb.append(t)
        vgb_sb = []
        for kc in range(nkc):
            t = vp.tile([P, D], dt.bfloat16, tag="vgb")
            nc.sync.dma_start(t[:], vgb[kc * P : (kc + 1) * P, :])
            vgb_sb.append(t)
        load_wqb(1)
        for p in range(2, NPAIR):
            load_wkb(p)
            load_wqb(p)
        wo_sb = []
        for pc in range(NCH):
            t = big.tile([P, D], dt.bfloat16, tag="wo")
            nc.sync.dma_start(t[:], wo[pc * P : (pc + 1) * P, :])
            wo_sb.append(t)

        # ---- work-unit emitters ---------------------------------------
        kt_t = [None] * NPAIR

        def emit_kt_half(p, half):
            # K projection of pair p, key half `half`, [d, s_k] layout
            if kt_t[p] is None:
                kt_t[p] = big.tile([P, SK], dt.bfloat16, tag="kt", name=f"kt{p}")
            hs = slice(half * SK2, (half + 1) * SK2)
            ps = psp.tile([P, 512], dt.float32, tag="proj", name=f"kt_ps{p}_{half}")
            for di in range(NCH):
                nc.tensor.matmul(
                    ps[:, 0:SK2],
                    wkb_sb[p][:, di * P : (di + 1) * P],
                    xk[di][:, hs],
                    start=(di == 0),
                    stop=(di == NCH - 1),
                )
            nc.vector.tensor_scalar_add(
                kt_t[p][:, hs], ps[:, 0:SK2], bk_sb[:, p : p + 1]
            )

        # V tiles: per head a 128-wide group [ones | 63 dead | 64 dims] so
        # the PV output puts the softmax denominator on PSUM partition 0
        # (read in place by reciprocal_approx_fast) and the dims at
        # partitions 64..127 (PSUM partition ranges cannot straddle the 64
        # line). Extra M is free: matmul cost is set by the moving free dim
        # N, not M.
        VW = 128
        vv_t = [None] * nkc

        def emit_vv(kc):
            t = vp.tile([P, H * VW], dt.bfloat16, tag="vv", name=f"vv{kc}")
            vv_t[kc] = t
            nc.gpsimd.memset(t[:], 1.0)
            for half in range(2):
                hs = slice(half * 512, (half + 1) * 512)
                ps = psp.tile([P, 512], dt.float32, tag="proj", name=f"v_ps{kc}_{half}")
                for di in range(NCH):
                    nc.tensor.matmul(
                        ps[:],
                        vgb_sb[kc][:, di * P : (di + 1) * P],
                        wv_sb[di][:, hs],
                        start=(di == 0),
                        stop=(di == NCH - 1),
                    )
                dst = t[:, half * 8 * VW : (half + 1) * 8 * VW].rearrange(
                    "p (h e) -> p h e", e=VW
                )[:, :, 64 : 64 + DK]
                srcv = ps[:].rearrange("p (h e) -> p h e", e=DK)
                nc.vector.tensor_copy(dst, srcv)

        qt_t = [None] * NPAIR

        def emit_qt_half(p, hq):
            if qt_t[p] is None:
                qt_t[p] = big.tile([P, S], dt.bfloat16, tag="qt", name=f"qt{p}")
            qs = slice(hq * 512, (hq + 1) * 512)
            ps = psp.tile([P, 512], dt.float32, tag="proj", name=f"qt_ps{p}_{hq}")
            for di in range(NCH):
                nc.tensor.matmul(
                    ps[:],
                    wqb_sb[p][:, di * P : (di + 1) * P],
                    xq[di][:, qs],
                    start=(di == 0),
                    stop=(di == NCH - 1),
                )
            nc.vector.tensor_scalar_add(qt_t[p][:, qs], ps[:], bq_sb[:, p : p + 1])

        ct_t = [None] * NPAIR

        def emit_oproj(qc, half):
            # output projection for query block qc, D-half `half` (+ bias)
            hs = slice(half * 512, (half + 1) * 512)
            ps = psp.tile([P, 512], dt.float32, tag="proj", name=f"o_ps{qc}_{half}")
            for pc in range(NCH):
                nc.tensor.matmul(
                    ps[:],
                    ct_t[pc][:, qc * P : (qc + 1) * P],
                    wo_sb[pc][:, hs],
                    start=(pc == 0),
                    stop=(pc == NCH - 1),
                )
            ob = strm.tile([P, 512], dt.bfloat16, tag="ob", bufs=OB_BUFS)
            nc.vector.tensor_add(ob[:], ps[:], bo_sb[:, hs])
            nc.sync.dma_start(out[qc * P : (qc + 1) * P, hs], ob[:])

        def emit_dummies(n, name):
            dps = psp.tile([P, 512], dt.float32, tag="proj", name=name)
            for _ in range(n):
                nc.tensor.matmul(
                    dps[:, 0:P],
                    scr[:, 0:P],
                    scr[:, 512 - P : 512],
                    start=True,
                    stop=True,
                )

        # ---- filler schedule ------------------------------------------
        # phase 1 (hq=0): pair p slots feed K/Q projections of pair p+1;
        # pair 7 slots start Q-half-1 projections for phase 2.
        # phase 2 (hq=1): slots drain a queue of remaining Q-half-1
        # projections and the O-projection of query blocks 0-3 (whose ct
        # rows completed in phase 1).
        p2_queue = []
        for i in range(6):
            p2_queue.append(("q1", i + 2))
            p2_queue.append(("o", i // 2, i % 2))
        oproj_done = set((u[1], u[2]) for u in p2_queue if u[0] == "o")
        oproj_done.update([(3, 0), (3, 1)])

        def filler(hq, p, kc):
            if hq == 0:
                if p == NPAIR - 1:
                    if kc == 0:
                        emit_qt_half(0, 1)
                    elif kc == 1:
                        emit_qt_half(1, 1)
                    elif kc == 2:
                        emit_dummies(4, f"dp1_{p}_{kc}")
                elif kc == 0:
                    emit_kt_half(p + 1, 0)
                elif kc == 1:
                    emit_kt_half(p + 1, 1)
                elif kc == 2:
                    emit_qt_half(p + 1, 0)
            else:
                if kc in (0, 2) and p2_queue:
                    u = p2_queue.pop(0)
                    if u[0] == "q1":
                        emit_qt_half(u[1], 1)
                    else:
                        emit_oproj(u[1], u[2])
                elif kc in (0, 2) and p == NPAIR - 1:
                    # no projection work left: keep the clock gate open
                    emit_dummies(4, f"d{p}_{kc}")

        # ---- pre-phase: K projections of pairs 0-1, Q half of pair 0 ---
        emit_kt_half(0, 0)
        emit_kt_half(0, 1)
        emit_kt_half(1, 0)
        emit_kt_half(1, 1)
        emit_qt_half(0, 0)

        # ---- attention: hq outer, pair inner ---------------------------
        for hq in range(2):
            for p in range(NPAIR):
                if ct_t[p] is None:
                    ct_t[p] = big.tile([P, S], dt.bfloat16, tag="ct", name=f"ct{p}")
                ct = ct_t[p]
                qs = slice(hq * 512, (hq + 1) * 512)
                pv0 = psp.tile([P, 512], dt.float32, tag="pv")
                pv1 = psp.tile([P, 512], dt.float32, tag="pv")
                for kc in range(nkc):
                    st = psp.tile([P, 1024], dt.float32, tag="st")
                    ks = slice(kc * P, (kc + 1) * P)
                    nc.tensor.matmul(
                        st[:, 0:512],
                        kt_t[p][0:DK, ks],
                        qt_t[p][0:DK, qs],
                        start=True,
                        stop=True,
                        tile_position=(0, 0),
                    )
                    nc.tensor.matmul(
                        st[:, 512:1024],
                        kt_t[p][DK:P, ks],
                        qt_t[p][DK:P, qs],
                        start=True,
                        stop=True,
                        tile_position=(DK, 0),
                    )
                    et = strm.tile([P, 1024], dt.bfloat16, tag="et", bufs=ET_BUFS)
                    nc.scalar.activation(
                        et[:], st[:], AF.Exp, bias=msk_sb[:, kc : kc + 1], scale=1.0
                    )
                    if hq == 0 and p == 0:
                        # JIT V projection: chunk kc right before its first use
                        emit_vv(kc)
                    else:
                        filler(hq, p, kc)
                    nc.tensor.matmul(
                        pv0[:],
                        vv_t[kc][:, (2 * p) * VW : (2 * p + 1) * VW],
                        et[:, 0:512],
                        start=(kc == 0),
                        stop=(kc == nkc - 1),
                    )
                    nc.tensor.matmul(
                        pv1[:],
                        vv_t[kc][:, (2 * p + 1) * VW : (2 * p + 2) * VW],
                        et[:, 512:1024],
                        start=(kc == 0),
                        stop=(kc == nkc - 1),
                    )
                if hq == 0 and p == 0:
                    # pair 0 used its slots for V; emit pair 1's Q here
                    emit_qt_half(1, 0)
                if hq == 1 and p == NPAIR - 1:
                    emit_oproj(3, 0)
                    emit_oproj(3, 1)
                for hloc, pv in ((0, pv0), (1, pv1)):
                    # pv partition 0 holds the softmax denominator; the dims
                    # sit at partitions 32..95 (32-aligned PSUM access)
                    rcp = strm.tile([1, 512], dt.float32, tag="rcp", bufs=CH_BUFS)
                    nc.vector.reciprocal_approx_fast(rcp[:], pv[0:1, :])
                    rb = strm.tile([DK, 512], dt.float32, tag="rb", bufs=CH_BUFS)
                    nc.gpsimd.partition_broadcast(rb[:], rcp[:])
                    nc.vector.tensor_mul(
                        ct[hloc * DK : (hloc + 1) * DK, qs], pv[64 : 64 + DK, :], rb[:]
                    )

        # ---- output projection tail (query blocks not drained early) ---
        for qc in range(NCH):
            for half in range(2):
                if (qc, half) not in oproj_done:
                    emit_oproj(qc, half)

    nc.finalize()
    return nc


def _band(w: np.ndarray, ncol: int) -> np.ndarray:
    # w: [1024, ncol*128]. Output row-block p holds column-band p rearranged
    # as [128 rows (r), 8 chunks (di) x 128]: out[p*128+r, di*128+c] =
    # w[di*128+r, p*128+c]  -- the stationary layout for lhsT slices.
    return np.ascontiguousarray(
        w.reshape(NCH, P, ncol, P).transpose(2, 1, 0, 3).reshape(ncol * P, D)
    )


def _make_in_maps(query, key, value, mask, Wq, bq, Wk, bk, Wv, bv, Wo, bo):
    query = np.asarray(query, dtype=np.float32)
    key = np.asarray(key, dtype=np.float32)
    value = np.asarray(value, dtype=np.float32)
    mask = np.asarray(mask)
    Wq = np.asarray(Wq, dtype=np.float32)
    Wk = np.asarray(Wk, dtype=np.float32)
    Wv = np.asarray(Wv, dtype=np.float32)
    Wo = np.asarray(Wo, dtype=np.float32)
    sc = np.float32(1.0 / math.sqrt(DK))
    bo_eff = (np.asarray(bv, np.float32) @ Wo + np.asarray(bo, np.float32)).reshape(
        1, D
    )

    idxs, nv = [], []
    for i in range(B):
        ix = np.nonzero(np.asarray(mask[i, 0]) != 0)[0]
        idxs.append(ix)
        nv.append(len(ix))
    nkc = min(NCH, max(1, -(-max(nv) // P)))
    SK = nkc * P

    bf16 = ml_dtypes.bfloat16
    wqb = _band(Wq * sc, NCH).astype(bf16)
    wkb = _band(Wk, NCH).astype(bf16)
    wv_b = np.ascontiguousarray(Wv).astype(bf16)
    wo_b = np.ascontiguousarray(Wo).astype(bf16)
    bq2 = np.ascontiguousarray((np.asarray(bq, np.float32) * sc).reshape(NCH, P).T)
    bk2 = np.ascontiguousarray(np.asarray(bk, np.float32).reshape(NCH, P).T)

    in_maps = []
    for i in range(B):
        ix = idxs[i]
        pad = SK - len(ix)
        ixp = np.concatenate([ix, np.zeros(pad, dtype=ix.dtype)])
        mb = np.full(SK, 0.0, dtype=np.float32)
        if pad:
            mb[len(ix) :] = NEGB
        kTg = np.ascontiguousarray(key[i][ixp].astype(bf16).T)
        vT = value[i][ixp].astype(bf16).T  # [D, SK]
        vgb = np.ascontiguousarray(_band(vT, nkc))
        in_maps.append(
            {
                "qT": np.ascontiguousarray(query[i].astype(bf16).T),
                "kTg": kTg,
                "vgb": vgb,
                "wqb": wqb,
                "wkb": wkb,
                "wv": wv_b,
                "wo": wo_b,
                "bq": bq2,
                "bk": bk2,
                "msk": np.ascontiguousarray(mb.reshape(nkc, P).T),
                "bo": bo_eff,
            }
        )
    return nkc, in_maps


def kernel(query, key, value, mask, Wq, bq, Wk, bk, Wv, bv, Wo, bo):
    nkc, in_maps = _make_in_maps(
        query, key, value, mask, Wq, bq, Wk, bk, Wv, bv, Wo, bo
    )
    if nkc not in _NC_CACHE:
        _NC_CACHE[nkc] = build_nc(nkc)
    nc = _NC_CACHE[nkc]
    res = run_bass_kernel_spmd(nc, in_maps, list(range(B)))
    return np.stack([res.results[i]["out"] for i in range(B)], axis=0).astype(
        np.float32
    )


# revision 10
# speedup vs baseline: 1.0340x; 1.0173x over previous
# Multi-headed attention (B=8, S=1024, D=1024, H=16) on 8 TRN2 NeuronCores.
# Strategy: pure batch data-parallel (one batch element per core, no
# collectives), all matmuls bf16 with fp32 PSUM accumulation.
#
# Structure (v2, hq-outer): attention runs query-half-outer / pair-inner so
# that the output projection of query blocks 0-3 overlaps the second
# attention half, shrinking the serial O-projection tail. All projection
# work (K pairs 1-7, V chunks, Q halves) is interleaved into the attention
# loops as PE filler so the scalar engine's exp stream starts ~12us in and
# stays covered; the HAM clock gate then never re-throttles the PE.
#
# Key optimizations:
#   - masked key positions are dropped on the host: key/value are gathered to
#     the unmasked positions (padded to a multiple of 128; an exp bias of
#     -30000 zeroes the pads exactly, matching the reference's -1e9 mask).
#     The program is compiled per padded chunk count (nkc) and cached.
#   - weights are pre-banded on the host so every DMA is a plain contiguous
#     [128, N] block transfer, emitted in consumption order.
#   - softmax denominators come free via a ones-column in the V tiles,
#     placed so the denominator lands on PSUM partition 0 where
#     reciprocal_approx_fast reads it in place (no row copy).
#   - dummy warmup matmuls cover the initial DMA window to open the HAM
#     activity window before real work.
#   - the output bias (bv @ Wo + bo) is added on-device during the output
#     projection evacuation; output is written back bf16 to halve the
#     final writeback DMA.
import math
import sys

sys.path.insert(0, "/opt/trn_rl_repo")

from contextlib import ExitStack

import ml_dtypes
import numpy as np

import concourse.bass as bass
import concourse.mybir as mybir
from concourse import bacc
from concourse import tile
from concourse.bass_utils import run_bass_kernel_spmd

dt = mybir.dt
AF = mybir.ActivationFunctionType

B, S, D, H, DK = 8, 1024, 1024, 16, 64
P = 128
NCH = D // P  # 8 chunks of 128 along the 1024-sized dims
NPAIR = H // 2  # 8 head pairs
NEGB = -30000.0  # exp underflows to exactly 0.0, matching the -1e9 masking

_NC_CACHE = {}


def build_nc(nkc: int):
    SK = nkc * P  # gathered+padded key length
    SK2 = SK // 2
    lean = nkc >= 7  # dense-mask fallback: shallower stream buffers
    ET_BUFS = 2 if lean else 4
    OB_BUFS = 2 if lean else 3
    CH_BUFS = 1 if lean else 2
    nc = bacc.Bacc()
    qT = nc.dram_tensor("qT", [D, S], dt.bfloat16, kind="ExternalInput")
    kTg = nc.dram_tensor("kTg", [D, SK], dt.bfloat16, kind="ExternalInput")
    vgb = nc.dram_tensor("vgb", [SK, D], dt.bfloat16, kind="ExternalInput")
    wqb = nc.dram_tensor("wqb", [D, D], dt.bfloat16, kind="ExternalInput")
    wkb = nc.dram_tensor("wkb", [D, D], dt.bfloat16, kind="ExternalInput")
    wv = nc.dram_tensor("wv", [D, D], dt.bfloat16, kind="ExternalInput")
    wo = nc.dram_tensor("wo", [D, D], dt.bfloat16, kind="ExternalInput")
    bq = nc.dram_tensor("bq", [P, NCH], dt.float32, kind="ExternalInput")
    bk = nc.dram_tensor("bk", [P, NCH], dt.float32, kind="ExternalInput")
    msk = nc.dram_tensor("msk", [P, nkc], dt.float32, kind="ExternalInput")
    bo = nc.dram_tensor("bo", [1, D], dt.float32, kind="ExternalInput")
    out = nc.dram_tensor("out", [S, D], dt.bfloat16, kind="ExternalOutput")

    with tile.TileContext(nc) as tc, ExitStack() as ctx:
        big = ctx.enter_context(tc.tile_pool(name="big", bufs=NCH))
        vp = ctx.enter_context(tc.tile_pool(name="vp", bufs=nkc))
        strm = ctx.enter_context(tc.tile_pool(name="strm", bufs=4))
        one = ctx.enter_context(tc.tile_pool(name="one", bufs=1))
        psp = ctx.enter_context(tc.tile_pool(name="psp", bufs=2, space="PSUM"))

        # ---- DMA emission in consumption order -------------------------
        wkb_sb = [None] * NPAIR

        def load_wkb(p):
            t = big.tile([P, D], dt.bfloat16, tag="wkb")
            nc.sync.dma_start(t[:], wkb[p * P : (p + 1) * P, :])
            wkb_sb[p] = t

        load_wkb(0)
        xk = []
        for di in range(NCH):
            t = big.tile([P, SK], dt.bfloat16, tag="xk")
            nc.sync.dma_start(t[:], kTg[di * P : (di + 1) * P, :])
            xk.append(t)
        for p in range(1, NPAIR):
            load_wkb(p)

        # PE warmup on a zeroed scratch tile: keeps the HAM activity window
        # busy while the first DMAs land so real work starts at 2.4 GHz.
        scr = one.tile([P, 512], dt.bfloat16, tag="scr")
        nc.gpsimd.memset(scr[:], 0.0)
        wps = psp.tile([P, 512], dt.float32, tag="proj")
        for _ in range(48):
            nc.tensor.matmul(
                wps[:, 0:P], scr[:, 0:P], scr[:, 512 - P : 512], start=True, stop=True
            )

        # small constants
        msk_sb = one.tile([P, nkc], dt.float32, tag="msk")
        nc.sync.dma_start(msk_sb[:], msk[:])
        bq_sb = one.tile([P, NCH], dt.float32, tag="bq")
        nc.sync.dma_start(bq_sb[:], bq[:])
        bk_sb = one.tile([P, NCH], dt.float32, tag="bk")
        nc.sync.dma_start(bk_sb[:], bk[:])
        bo_row = one.tile([1, D], dt.float32, tag="bo_row")
        nc.sync.dma_start(bo_row[:], bo[:])

        # warm the ACT exp table while DMAs stream
        warm = one.tile([1, nkc], dt.float32, tag="warm")
        nc.scalar.activation(warm[:], msk_sb[0:1, :], AF.Exp, bias=0.0, scale=1.0)

        # output-bias row broadcast to all partitions
        bo_sb = one.tile([P, D], dt.float32, tag="bo_sb")
        nc.gpsimd.partition_broadcast(bo_sb[:], bo_row[:])

        # remaining loads, in consumption order
        wqb_sb = [None] * NPAIR

        def load_wqb(p):
            t = big.tile([P, D], dt.bfloat16, tag="wqb")
            nc.sync.dma_start(t[:], wqb[p * P : (p + 1) * P, :])
            wqb_sb[p] = t

        wv_sb = []
        for di in range(NCH):
            t = big.tile([P, D], dt.bfloat16, tag="wv")
            nc.sync.dma_start(t[:], wv[di * P : (di + 1) * P, :])
            wv_sb.append(t)
        vgb_sb = []
        for kc in range(nkc):
            t = vp.tile([P, D], dt.bfloat16, tag="vgb")
            nc.sync.dma_start(t[:], vgb[kc * P : (kc + 1) * P, :])
            vgb_sb.append(t)
        load_wqb(0)
        load_wqb(1)
        xq = []
        for di in range(NCH):
            t = big.tile([P, S], dt.bfloat16, tag="xq")
            nc.sync.dma_start(t[:], qT[di * P : (di + 1) * P, :])
            xq.append(t)
        for p in range(2, NPAIR):
            load_wqb(p)
        wo_sb = []
        for pc in range(NCH):
            t = big.tile([P, D], dt.bfloat16, tag="wo")
            nc.sync.dma_start(t[:], wo[pc * P : (pc + 1) * P, :])
            wo_sb.append(t)

        # ---- work-unit emitters ---------------------------------------
        kt_t = [None] * NPAIR

        def emit_kt_half(p, half):
            # K projection of pair p, key half `half`, [d, s_k] layout
            if kt_t[p] is None:
                kt_t[p] = big.tile([P, SK], dt.bfloat16, tag="kt", name=f"kt{p}")
            hs = slice(half * SK2, (half + 1) * SK2)
            ps = psp.tile([P, 512], dt.float32, tag="proj", name=f"kt_ps{p}_{half}")
            for di in range(NCH):
                nc.tensor.matmul(
                    ps[:, 0:SK2],
                    wkb_sb[p][:, di * P : (di + 1) * P],
                    xk[di][:, hs],
                    start=(di == 0),
                    stop=(di == NCH - 1),
                )
            nc.vector.tensor_scalar_add(
                kt_t[p][:, hs], ps[:, 0:SK2], bk_sb[:, p : p + 1]
            )

        # V tiles: per head a 128-wide group [ones | 63 dead | 64 dims] so
        # the PV output puts the softmax denominator on PSUM partition 0
        # (read in place by reciprocal_approx_fast) and the dims at
        # partitions 64..127 (PSUM partition ranges cannot straddle the 64
        # line). Extra M is free: matmul cost is set by the moving free dim
        # N, not M.
        VW = 128
        vv_t = [None] * nkc

        def emit_vv(kc):
            t = vp.tile([P, H * VW], dt.bfloat16, tag="vv", name=f"vv{kc}")
            vv_t[kc] = t
            nc.gpsimd.memset(t[:], 1.0)
            for half in range(2):
                hs = slice(half * 512, (half + 1) * 512)
                ps = psp.tile([P, 512], dt.float32, tag="proj", name=f"v_ps{kc}_{half}")
                for di in range(NCH):
                    nc.tensor.matmul(
                        ps[:],
                        vgb_sb[kc][:, di * P : (di + 1) * P],
                        wv_sb[di][:, hs],
                        start=(di == 0),
                        stop=(di == NCH - 1),
                    )
                dst = t[:, half * 8 * VW : (half + 1) * 8 * VW].rearrange(
                    "p (h e) -> p h e", e=VW
                )[:, :, 64 : 64 + DK]
                srcv = ps[:].rearrange("p (h e) -> p h e", e=DK)
                nc.vector.tensor_copy(dst, srcv)

        qt_t = [None] * NPAIR

        def emit_qt_half(p, hq):
            if qt_t[p] is None:
                qt_t[p] = big.tile([P, S], dt.bfloat16, tag="qt", name=f"qt{p}")
            qs = slice(hq * 512, (hq + 1) * 512)
            ps = psp.tile([P, 512], dt.float32, tag="proj", name=f"qt_ps{p}_{hq}")
            for di in range(NCH):
                nc.tensor.matmul(
                    ps[:],
                    wqb_sb[p][:, di * P : (di + 1) * P],
                    xq[di][:, qs],
                    start=(di == 0),
                    stop=(di == NCH - 1),
                )
            nc.vector.tensor_scalar_add(qt_t[p][:, qs], ps[:], bq_sb[:, p : p + 1])

        ct_t = [None] * NPAIR

        def emit_oproj(qc, half):
            # output projection for query block qc, D-half `half` (+ bias)
            hs = slice(half * 512, (half + 1) * 512)
            ps = psp.tile([P, 512], dt.float32, tag="proj", name=f"o_ps{qc}_{half}")
            for pc in range(NCH):
                nc.tensor.matmul(
                    ps[:],
                    ct_t[pc][:, qc * P : (qc + 1) * P],
                    wo_sb[pc][:, hs],
                    start=(pc == 0),
                    stop=(pc == NCH - 1),
                )
            ob = strm.tile([P, 512], dt.bfloat16, tag="ob", bufs=OB_BUFS)
            nc.vector.tensor_add(ob[:], ps[:], bo_sb[:, hs])
            nc.sync.dma_start(out[qc * P : (qc + 1) * P, hs], ob[:])

        def emit_dummies(n, name):
            dps = psp.tile([P, 512], dt.float32, tag="proj", name=name)
            for _ in range(n):
                nc.tensor.matmul(
                    dps[:, 0:P],
                    scr[:, 0:P],
                    scr[:, 512 - P : 512],
                    start=True,
                    stop=True,
                )

        # ---- filler schedule ------------------------------------------
        # phase 1 (hq=0): pair p slots feed K/Q projections of pair p+1;
        # pair 7 slots start Q-half-1 projections for phase 2.
        # phase 2 (hq=1): slots drain a queue of remaining Q-half-1
        # projections and the O-projection of query blocks 0-3 (whose ct
        # rows completed in phase 1).
        p2_queue = []
        for i in range(6):
            p2_queue.append(("q1", i + 2))
            p2_queue.append(("o", i // 2, i % 2))
        oproj_done = set((u[1], u[2]) for u in p2_queue if u[0] == "o")
        oproj_done.update([(3, 0), (3, 1)])

        def filler(hq, p, kc):
            if hq == 0:
                # one Q-projection half per pair keeps the PE fed while the
                # scalar engine grinds exp (phase 1 is ACT-bound)
                if kc == 0:
                    if p < NPAIR - 2:
                        emit_qt_half(p + 2, 0)
                    else:
                        emit_qt_half(p - (NPAIR - 2), 1)
            else:
                if kc in (0, 2) and p2_queue:
                    u = p2_queue.pop(0)
                    if u[0] == "q1":
                        emit_qt_half(u[1], 1)
                    else:
                        emit_oproj(u[1], u[2])
                elif kc in (0, 2) and p == NPAIR - 1:
                    # no projection work left: keep the clock gate open
                    emit_dummies(4, f"d{p}_{kc}")

        # ---- pre-phase (DMA-paced): all K projections, all V chunks,
        # then the first two Q halves. ~40us of dense PE work at the rate
        # the DMA engines actually deliver the 12.5MB of inputs (~190GB/s).
        for p in range(NPAIR):
            emit_kt_half(p, 0)
            emit_kt_half(p, 1)
        for kc in range(nkc):
            emit_vv(kc)
        emit_qt_half(0, 0)
        emit_qt_half(1, 0)

        # ---- attention: hq outer, pair inner ---------------------------
        for hq in range(2):
            for p in range(NPAIR):
                if ct_t[p] is None:
                    ct_t[p] = big.tile([P, S], dt.bfloat16, tag="ct", name=f"ct{p}")
                ct = ct_t[p]
                qs = slice(hq * 512, (hq + 1) * 512)
                pv0 = psp.tile([P, 512], dt.float32, tag="pv")
                pv1 = psp.tile([P, 512], dt.float32, tag="pv")
                for kc in range(nkc):
                    st = psp.tile([P, 1024], dt.float32, tag="st")
                    ks = slice(kc * P, (kc + 1) * P)
                    nc.tensor.matmul(
                        st[:, 0:512],
                        kt_t[p][0:DK, ks],
                        qt_t[p][0:DK, qs],
                        start=True,
                        stop=True,
                        tile_position=(0, 0),
                    )
                    nc.tensor.matmul(
                        st[:, 512:1024],
                        kt_t[p][DK:P, ks],
                        qt_t[p][DK:P, qs],
                        start=True,
                        stop=True,
                        tile_position=(DK, 0),
                    )
                    et = strm.tile([P, 1024], dt.bfloat16, tag="et", bufs=ET_BUFS)
                    nc.scalar.activation(
                        et[:], st[:], AF.Exp, bias=msk_sb[:, kc : kc + 1], scale=1.0
                    )
                    filler(hq, p, kc)
                    nc.tensor.matmul(
                        pv0[:],
                        vv_t[kc][:, (2 * p) * VW : (2 * p + 1) * VW],
                        et[:, 0:512],
                        start=(kc == 0),
                        stop=(kc == nkc - 1),
                    )
                    nc.tensor.matmul(
                        pv1[:],
                        vv_t[kc][:, (2 * p + 1) * VW : (2 * p + 2) * VW],
                        et[:, 512:1024],
                        start=(kc == 0),
                        stop=(kc == nkc - 1),
                    )
                if hq == 1 and p == NPAIR - 1:
                    emit_oproj(3, 0)
                    emit_oproj(3, 1)
                for hloc, pv in ((0, pv0), (1, pv1)):
                    # pv partition 0 holds the softmax denominator; the dims
                    # sit at partitions 32..95 (32-aligned PSUM access)
                    rcp = strm.tile([1, 512], dt.float32, tag="rcp", bufs=CH_BUFS)
                    nc.vector.reciprocal_approx_fast(rcp[:], pv[0:1, :])
                    rb = strm.tile([DK, 512], dt.float32, tag="rb", bufs=CH_BUFS)
                    nc.gpsimd.partition_broadcast(rb[:], rcp[:])
                    nc.vector.tensor_mul(
                        ct[hloc * DK : (hloc + 1) * DK, qs], pv[64 : 64 + DK, :], rb[:]
                    )

        # ---- output projection tail (query blocks not drained early) ---
        for qc in range(NCH):
            for half in range(2):
                if (qc, half) not in oproj_done:
                    emit_oproj(qc, half)

    nc.finalize()
    return nc


def _band(w: np.ndarray, ncol: int) -> np.ndarray:
    # w: [1024, ncol*128]. Output row-block p holds column-band p rearranged
    # as [128 rows (r), 8 chunks (di) x 128]: out[p*128+r, di*128+c] =
    # w[di*128+r, p*128+c]  -- the stationary layout for lhsT slices.
    return np.ascontiguousarray(
        w.reshape(NCH, P, ncol, P).transpose(2, 1, 0, 3).reshape(ncol * P, D)
    )


def _make_in_maps(query, key, value, mask, Wq, bq, Wk, bk, Wv, bv, Wo, bo):
    query = np.asarray(query, dtype=np.float32)
    key = np.asarray(key, dtype=np.float32)
    value = np.asarray(value, dtype=np.float32)
    mask = np.asarray(mask)
    Wq = np.asarray(Wq, dtype=np.float32)
    Wk = np.asarray(Wk, dtype=np.float32)
    Wv = np.asarray(Wv, dtype=np.float32)
    Wo = np.asarray(Wo, dtype=np.float32)
    sc = np.float32(1.0 / math.sqrt(DK))
    bo_eff = (np.asarray(bv, np.float32) @ Wo + np.asarray(bo, np.float32)).reshape(
        1, D
    )

    idxs, nv = [], []
    for i in range(B):
        ix = np.nonzero(np.asarray(mask[i, 0]) != 0)[0]
        idxs.append(ix)
        nv.append(len(ix))
    nkc = min(NCH, max(1, -(-max(nv) // P)))
    SK = nkc * P

    bf16 = ml_dtypes.bfloat16
    wqb = _band(Wq * sc, NCH).astype(bf16)
    wkb = _band(Wk, NCH).astype(bf16)
    wv_b = np.ascontiguousarray(Wv).astype(bf16)
    wo_b = np.ascontiguousarray(Wo).astype(bf16)
    bq2 = np.ascontiguousarray((np.asarray(bq, np.float32) * sc).reshape(NCH, P).T)
    bk2 = np.ascontiguousarray(np.asarray(bk, np.float32).reshape(NCH, P).T)

    in_maps = []
    for i in range(B):
        ix = idxs[i]
        pad = SK - len(ix)
        ixp = np.concatenate([ix, np.zeros(pad, dtype=ix.dtype)])
        mb = np.full(SK, 0.0, dtype=np.float32)
        if pad:
            mb[len(ix) :] = NEGB
        kTg = np.ascontiguousarray(key[i][ixp].astype(bf16).T)
        vT = value[i][ixp].astype(bf16).T  # [D, SK]
        vgb = np.ascontiguousarray(_band(vT, nkc))
        in_maps.append(
            {
                "qT": np.ascontiguousarray(query[i].astype(bf16).T),
                "kTg": kTg,
                "vgb": vgb,
                "wqb": wqb,
                "wkb": wkb,
                "wv": wv_b,
                "wo": wo_b,
                "bq": bq2,
                "bk": bk2,
                "msk": np.ascontiguousarray(mb.reshape(nkc, P).T),
                "bo": bo_eff,
            }
        )
    return nkc, in_maps


def kernel(query, key, value, mask, Wq, bq, Wk, bk, Wv, bv, Wo, bo):
    nkc, in_maps = _make_in_maps(
        query, key, value, mask, Wq, bq, Wk, bk, Wv, bv, Wo, bo
    )
    if nkc not in _NC_CACHE:
        _NC_CACHE[nkc] = build_nc(nkc)
    nc = _NC_CACHE[nkc]
    res = run_bass_kernel_spmd(nc, in_maps, list(range(B)))
    return np.stack([res.results[i]["out"] for i in range(B)], axis=0).astype(
        np.float32
    )


# revision 14
# speedup vs baseline: 1.0735x; 1.0381x over previous
# Multi-headed attention (B=8, S=1024, D=1024, H=16) on 8 TRN2 NeuronCores.
# Strategy: pure batch data-parallel (one batch element per core, no
# collectives), all matmuls bf16 with fp32 PSUM accumulation.
#
# Structure (v2, hq-outer): attention runs query-half-outer / pair-inner so
# that the output projection of query blocks 0-3 overlaps the second
# attention half, shrinking the serial O-projection tail. All projection
# work (K pairs 1-7, V chunks, Q halves) is interleaved into the attention
# loops as PE filler so the scalar engine's exp stream starts ~12us in and
# stays covered; the HAM clock gate then never re-throttles the PE.
#
# Key optimizations:
#   - masked key positions are dropped on the host: key/value are gathered to
#     the unmasked positions (padded to a multiple of 128; an exp bias of
#     -30000 zeroes the pads exactly, matching the reference's -1e9 mask).
#     The program is compiled per padded chunk count (nkc) and cached.
#   - weights are pre-banded on the host so every DMA is a plain contiguous
#     [128, N] block transfer, emitted in consumption order.
#   - softmax denominators come free via a ones-column in the V tiles,
#     placed so the denominator lands on PSUM partition 0 where
#     reciprocal_approx_fast reads it in place (no row copy).
#   - dummy warmup matmuls cover the initial DMA window to open the HAM
#     activity window before real work.
#   - the output bias (bv @ Wo + bo) is added on-device during the output
#     projection evacuation; output is written back bf16 to halve the
#     final writeback DMA.
import math
import sys

sys.path.insert(0, "/opt/trn_rl_repo")

from contextlib import ExitStack

import ml_dtypes
import numpy as np

import concourse.bass as bass
import concourse.mybir as mybir
from concourse import bacc
from concourse import tile
from concourse.bass_utils import run_bass_kernel_spmd

dt = mybir.dt
AF = mybir.ActivationFunctionType

B, S, D, H, DK = 8, 1024, 1024, 16, 64
P = 128
NCH = D // P  # 8 chunks of 128 along the 1024-sized dims
NPAIR = H // 2  # 8 head pairs
NEGB = -30000.0  # exp underflows to exactly 0.0, matching the -1e9 masking

_NC_CACHE = {}


def build_nc(nkc: int):
    SK = nkc * P  # gathered+padded key length
    SK2 = SK // 2
    lean = nkc >= 7  # dense-mask fallback: shallower stream buffers
    ET_BUFS = 2 if lean else 4
    OB_BUFS = 2 if lean else 3
    CH_BUFS = 1 if lean else 2
    nc = bacc.Bacc()
    qT = nc.dram_tensor("qT", [D, S], dt.bfloat16, kind="ExternalInput")
    kTg = nc.dram_tensor("kTg", [D, SK], dt.bfloat16, kind="ExternalInput")
    vgb = nc.dram_tensor("vgb", [SK, D], dt.bfloat16, kind="ExternalInput")
    wqb = nc.dram_tensor("wqb", [D, D], dt.bfloat16, kind="ExternalInput")
    wkb = nc.dram_tensor("wkb", [D, D], dt.bfloat16, kind="ExternalInput")
    wv = nc.dram_tensor("wv", [D, D], dt.bfloat16, kind="ExternalInput")
    wo = nc.dram_tensor("wo", [D, D], dt.bfloat16, kind="ExternalInput")
    bq = nc.dram_tensor("bq", [P, NCH], dt.float32, kind="ExternalInput")
    bk = nc.dram_tensor("bk", [P, NCH], dt.float32, kind="ExternalInput")
    msk = nc.dram_tensor("msk", [P, nkc], dt.float32, kind="ExternalInput")
    bo = nc.dram_tensor("bo", [1, D], dt.float32, kind="ExternalInput")
    out = nc.dram_tensor("out", [S, D], dt.bfloat16, kind="ExternalOutput")

    with tile.TileContext(nc) as tc, ExitStack() as ctx:
        big = ctx.enter_context(tc.tile_pool(name="big", bufs=NCH))
        vp = ctx.enter_context(tc.tile_pool(name="vp", bufs=nkc))
        strm = ctx.enter_context(tc.tile_pool(name="strm", bufs=4))
        one = ctx.enter_context(tc.tile_pool(name="one", bufs=1))
        psp = ctx.enter_context(tc.tile_pool(name="psp", bufs=2, space="PSUM"))

        # ---- DMA emission in consumption order -------------------------
        wkb_sb = [None] * NPAIR

        def load_wkb(p):
            t = big.tile([P, D], dt.bfloat16, tag="wkb")
            nc.sync.dma_start(t[:], wkb[p * P : (p + 1) * P, :])
            wkb_sb[p] = t

        load_wkb(0)
        xk = []
        for di in range(NCH):
            t = big.tile([P, SK], dt.bfloat16, tag="xk")
            nc.sync.dma_start(t[:], kTg[di * P : (di + 1) * P, :])
            xk.append(t)
        for p in range(1, NPAIR):
            load_wkb(p)

        # PE warmup on a zeroed scratch tile: keeps the HAM activity window
        # busy while the first DMAs land so real work starts at 2.4 GHz.
        scr = one.tile([P, 512], dt.bfloat16, tag="scr")
        nc.gpsimd.memset(scr[:], 0.0)
        wps = psp.tile([P, 512], dt.float32, tag="proj")
        for _ in range(48):
            nc.tensor.matmul(
                wps[:, 0:P], scr[:, 0:P], scr[:, 512 - P : 512], start=True, stop=True
            )

        # small constants
        msk_sb = one.tile([P, nkc], dt.float32, tag="msk")
        nc.sync.dma_start(msk_sb[:], msk[:])
        bq_sb = one.tile([P, NCH], dt.float32, tag="bq")
        nc.sync.dma_start(bq_sb[:], bq[:])
        bk_sb = one.tile([P, NCH], dt.float32, tag="bk")
        nc.sync.dma_start(bk_sb[:], bk[:])
        bo_row = one.tile([1, D], dt.float32, tag="bo_row")
        nc.sync.dma_start(bo_row[:], bo[:])

        # warm the ACT exp table while DMAs stream
        warm = one.tile([1, nkc], dt.float32, tag="warm")
        nc.scalar.activation(warm[:], msk_sb[0:1, :], AF.Exp, bias=0.0, scale=1.0)

        # output-bias row broadcast to all partitions
        bo_sb = one.tile([P, D], dt.float32, tag="bo_sb")
        nc.gpsimd.partition_broadcast(bo_sb[:], bo_row[:])

        # remaining loads, in consumption order
        wqb_sb = [None] * NPAIR

        def load_wqb(p):
            t = big.tile([P, D], dt.bfloat16, tag="wqb")
            nc.sync.dma_start(t[:], wqb[p * P : (p + 1) * P, :])
            wqb_sb[p] = t

        wv_sb = []
        for di in range(NCH):
            t = big.tile([P, D], dt.bfloat16, tag="wv")
            nc.sync.dma_start(t[:], wv[di * P : (di + 1) * P, :])
            wv_sb.append(t)
        vgb_sb = [None] * nkc

        def load_vgb(kc):
            t = vp.tile([P, D], dt.bfloat16, tag="vgb")
            nc.sync.dma_start(t[:], vgb[kc * P : (kc + 1) * P, :])
            vgb_sb[kc] = t

        NVF = min(3, nkc)  # V chunks projected in the front phase
        for kc in range(NVF):
            load_vgb(kc)
        load_wqb(0)
        load_wqb(1)
        xq = []
        for di in range(NCH):
            t = big.tile([P, S], dt.bfloat16, tag="xq")
            nc.sync.dma_start(t[:], qT[di * P : (di + 1) * P, :])
            xq.append(t)
        for kc in range(NVF, nkc):
            load_vgb(kc)
        for p in range(2, NPAIR):
            load_wqb(p)
        wo_sb = []
        for pc in range(NCH):
            t = big.tile([P, D], dt.bfloat16, tag="wo")
            nc.sync.dma_start(t[:], wo[pc * P : (pc + 1) * P, :])
            wo_sb.append(t)

        # ---- work-unit emitters ---------------------------------------
        kt_t = [None] * NPAIR

        def emit_kt_half(p, half):
            # K projection of pair p, key half `half`, [d, s_k] layout
            if kt_t[p] is None:
                kt_t[p] = big.tile([P, SK], dt.bfloat16, tag="kt", name=f"kt{p}")
            hs = slice(half * SK2, (half + 1) * SK2)
            ps = psp.tile([P, 512], dt.float32, tag="proj", name=f"kt_ps{p}_{half}")
            for di in range(NCH):
                nc.tensor.matmul(
                    ps[:, 0:SK2],
                    wkb_sb[p][:, di * P : (di + 1) * P],
                    xk[di][:, hs],
                    start=(di == 0),
                    stop=(di == NCH - 1),
                )
            nc.vector.tensor_scalar_add(
                kt_t[p][:, hs], ps[:, 0:SK2], bk_sb[:, p : p + 1]
            )

        # V tiles: per head a 128-wide group [ones | 63 dead | 64 dims] so
        # the PV output puts the softmax denominator on PSUM partition 0
        # (readable in place by reciprocal_approx_fast) and the dims at
        # partitions 64..127 (PSUM partition ranges cannot straddle the 64
        # line). The extra M is free: matmul cost is set by the moving free
        # dim N, not M.
        VW = P
        vv_t = [None] * nkc

        def emit_vv(kc, half=None):
            if vv_t[kc] is None:
                t = vp.tile([P, H * VW], dt.bfloat16, tag="vv", name=f"vv{kc}")
                vv_t[kc] = t
                nc.gpsimd.memset(t[:], 1.0)
            t = vv_t[kc]
            halves = (0, 1) if half is None else (half,)
            for hf in halves:
                hs = slice(hf * 512, (hf + 1) * 512)
                ps = psp.tile([P, 512], dt.float32, tag="proj", name=f"v_ps{kc}_{hf}")
                for di in range(NCH):
                    nc.tensor.matmul(
                        ps[:],
                        vgb_sb[kc][:, di * P : (di + 1) * P],
                        wv_sb[di][:, hs],
                        start=(di == 0),
                        stop=(di == NCH - 1),
                    )
                dst = t[:, hf * 8 * VW : (hf + 1) * 8 * VW].rearrange(
                    "p (h e) -> p h e", e=VW
                )[:, :, 64 : 64 + DK]
                srcv = ps[:].rearrange("p (h e) -> p h e", e=DK)
                nc.vector.tensor_copy(dst, srcv)

        qt_t = [None] * NPAIR

        def emit_qt_quarter(p, quarter):
            if qt_t[p] is None:
                qt_t[p] = big.tile([P, S], dt.bfloat16, tag="qt", name=f"qt{p}")
            qs = slice(quarter * 256, (quarter + 1) * 256)
            ps = psp.tile([P, 512], dt.float32, tag="proj", name=f"qt_ps{p}_{quarter}")
            for di in range(NCH):
                nc.tensor.matmul(
                    ps[:, 0:256],
                    wqb_sb[p][:, di * P : (di + 1) * P],
                    xq[di][:, qs],
                    start=(di == 0),
                    stop=(di == NCH - 1),
                )
            nc.vector.tensor_scalar_add(
                qt_t[p][:, qs], ps[:, 0:256], bq_sb[:, p : p + 1]
            )

        def emit_qt_half(p, hq):
            emit_qt_quarter(p, 2 * hq)
            emit_qt_quarter(p, 2 * hq + 1)

        ct_t = [None] * NPAIR
        o_ps = {}

        def emit_oproj_sub(qc, half, sub):
            # output projection for query block qc, D-half `half`, pair
            # sub-group `sub` (pairs 0-3 / 4-7); bias + writeback on sub 1
            hs = slice(half * 512, (half + 1) * 512)
            if sub == 0:
                o_ps[(qc, half)] = psp.tile(
                    [P, 512], dt.float32, tag="proj", name=f"o_ps{qc}_{half}"
                )
            ps = o_ps[(qc, half)]
            for pc in range(4 * sub, 4 * sub + 4):
                nc.tensor.matmul(
                    ps[:],
                    ct_t[pc][:, qc * P : (qc + 1) * P],
                    wo_sb[pc][:, hs],
                    start=(pc == 0),
                    stop=(pc == NCH - 1),
                )
            if sub == 1:
                ob = strm.tile([P, 512], dt.bfloat16, tag="ob", bufs=OB_BUFS)
                nc.vector.tensor_add(ob[:], ps[:], bo_sb[:, hs])
                nc.sync.dma_start(out[qc * P : (qc + 1) * P, hs], ob[:])

        def emit_oproj(qc, half):
            emit_oproj_sub(qc, half, 0)
            emit_oproj_sub(qc, half, 1)

        def emit_dummies(n, name):
            dps = psp.tile([P, 512], dt.float32, tag="proj", name=name)
            for _ in range(n):
                nc.tensor.matmul(
                    dps[:, 0:P],
                    scr[:, 0:P],
                    scr[:, 512 - P : 512],
                    start=True,
                    stop=True,
                )

        # ---- filler queues --------------------------------------------
        # Attention is ACT(exp)-bound: each (pair, query-half) unit costs
        # ~5.7us of exp on the scalar engine vs ~3.2us of scores+PV on the
        # PE. The leftover PE capacity runs "filler" units popped from a
        # queue: remaining V chunks, Q projection quarters, and in phase 2
        # the O projection of query blocks 0-3 (whose ct rows completed in
        # phase 1). Units are small (~0.9-1.8us) so the 2-deep st ring can
        # absorb the delay they add before the next scores issue.
        f_q = [[], []]
        for kc in range(NVF, nkc):
            f_q[0] += [("v", kc, 0), ("v", kc, 1)]
        for p in range(2, NPAIR):
            f_q[0] += [("q", p, 0), ("q", p, 1)]
        f_q[0] += [("q", 0, 2), ("q", 0, 3), ("q", 1, 2), ("q", 1, 3)]
        for p in range(2, NPAIR):
            f_q[1] += [("q", p, 2), ("q", p, 3)]
            qc = p - 2
            if qc < 4:
                f_q[1] += [("o", qc, 0), ("o", qc, 1)]
        oproj_done = set((u[1], u[2]) for u in f_q[1] if u[0] == "o")

        def pop_filler(phase):
            if not f_q[phase]:
                return False
            u = f_q[phase].pop(0)
            if u[0] == "q":
                emit_qt_quarter(u[1], u[2])
            elif u[0] == "v":
                emit_vv(u[1], half=u[2])
            else:
                emit_oproj(u[1], u[2])
            return True

        # ---- pre-phase (DMA-paced): K projections, first V chunks, the
        # first two Q halves. The DMA queue only starts delivering ~9us in
        # and ramps to ~300GB/s; this front consumes exactly at that pace.
        for p in range(NPAIR):
            emit_kt_half(p, 0)
            emit_kt_half(p, 1)
        for kc in range(NVF):
            emit_vv(kc)
        emit_qt_half(0, 0)
        emit_qt_half(1, 0)

        # ---- attention: software-pipelined over (hq, pair, kc) ---------
        # The PE queue is in-order, and PV(kc) blocks on exp(kc); emitting
        # scores LA positions ahead of PV keeps completed score tiles
        # queued for the scalar engine so the exp stream never starves.
        units = [(hq, p) for hq in range(2) for p in range(NPAIR)]
        SPOS = [(ui, kc) for ui in range(len(units)) for kc in range(nkc)]
        LA = min(3, nkc)
        pvs = {}
        sts = {}

        for p in range(NPAIR):
            ct_t[p] = big.tile([P, S], dt.bfloat16, tag="ct", name=f"ct{p}")

        def emit_sc(ui, kc):
            hq, p = units[ui]
            qs = slice(hq * 512, (hq + 1) * 512)
            st = psp.tile([P, 1024], dt.float32, tag="st")
            sts[(ui, kc)] = st
            ks = slice(kc * P, (kc + 1) * P)
            nc.tensor.matmul(
                st[:, 0:512],
                kt_t[p][0:DK, ks],
                qt_t[p][0:DK, qs],
                start=True,
                stop=True,
                tile_position=(0, 0),
            )
            nc.tensor.matmul(
                st[:, 512:1024],
                kt_t[p][DK:P, ks],
                qt_t[p][DK:P, qs],
                start=True,
                stop=True,
                tile_position=(DK, 0),
            )
            et = strm.tile([P, 1024], dt.bfloat16, tag="et", bufs=ET_BUFS)
            nc.scalar.activation(
                et[:], st[:], AF.Exp, bias=msk_sb[:, kc : kc + 1], scale=1.0
            )
            sts[(ui, kc)] = et

        def emit_pv(ui, kc):
            hq, p = units[ui]
            if kc == 0:
                pvs[ui] = (
                    psp.tile([P, 512], dt.float32, tag="pv", name=f"pv0_{ui}"),
                    psp.tile([P, 512], dt.float32, tag="pv", name=f"pv1_{ui}"),
                )
            pv0, pv1 = pvs[ui]
            et = sts.pop((ui, kc))
            for hloc, pv in ((0, pv0), (1, pv1)):
                nc.tensor.matmul(
                    pv[:],
                    vv_t[kc][:, (2 * p + hloc) * VW : (2 * p + hloc + 1) * VW],
                    et[:, hloc * 512 : (hloc + 1) * 512],
                    start=(kc == 0),
                    stop=(kc == nkc - 1),
                )

        def emit_norm(ui):
            hq, p = units[ui]
            qs = slice(hq * 512, (hq + 1) * 512)
            pv0, pv1 = pvs.pop(ui)
            for hloc, pv in ((0, pv0), (1, pv1)):
                # evacuate the whole pv tile (den at partition 0, dims at
                # 64..127) to SBUF in one DVE op -- this frees the PSUM
                # bank so the next unit's PV can start; the rest of the
                # softmax-normalize chain runs out of SBUF off that ring
                pvc = strm.tile([P, 512], dt.float32, tag="pvc", bufs=ET_BUFS)
                nc.vector.tensor_copy(pvc[:], pv[:])
                rcp = strm.tile([1, 512], dt.float32, tag="rcp", bufs=CH_BUFS)
                nc.vector.reciprocal_approx_fast(rcp[:], pvc[0:1, :])
                rb = strm.tile([P, 512], dt.float32, tag="rb", bufs=CH_BUFS)
                nc.gpsimd.partition_broadcast(rb[:], rcp[:])
                nc.vector.tensor_mul(
                    ct_t[p][hloc * DK : (hloc + 1) * DK, qs],
                    pvc[64:P, :],
                    rb[64:P, :],
                )

        for i, (ui, kc) in enumerate(SPOS):
            emit_sc(ui, kc)
            j = i - LA
            if j >= 0:
                uj, kj = SPOS[j]
                emit_pv(uj, kj)
                if kj == nkc - 1:
                    emit_norm(uj)
            if kc >= 1 and kc <= 3:
                pop_filler(units[ui][0])
        for j in range(len(SPOS) - LA, len(SPOS)):
            uj, kj = SPOS[j]
            emit_pv(uj, kj)
            if kj == nkc - 1:
                emit_norm(uj)

        # ---- output projection tail (query blocks 4-7 + leftovers) -----
        while pop_filler(0) or pop_filler(1):
            pass
        for qc in range(NCH):
            for half in range(2):
                if (qc, half) not in oproj_done:
                    emit_oproj(qc, half)

    nc.finalize()
    return nc


def _band(w: np.ndarray, ncol: int) -> np.ndarray:
    # w: [1024, ncol*128]. Output row-block p holds column-band p rearranged
    # as [128 rows (r), 8 chunks (di) x 128]: out[p*128+r, di*128+c] =
    # w[di*128+r, p*128+c]  -- the stationary layout for lhsT slices.
    return np.ascontiguousarray(
        w.reshape(NCH, P, ncol, P).transpose(2, 1, 0, 3).reshape(ncol * P, D)
    )


def _make_in_maps(query, key, value, mask, Wq, bq, Wk, bk, Wv, bv, Wo, bo):
    query = np.asarray(query, dtype=np.float32)
    key = np.asarray(key, dtype=np.float32)
    value = np.asarray(value, dtype=np.float32)
    mask = np.asarray(mask)
    Wq = np.asarray(Wq, dtype=np.float32)
    Wk = np.asarray(Wk, dtype=np.float32)
    Wv = np.asarray(Wv, dtype=np.float32)
    Wo = np.asarray(Wo, dtype=np.float32)
    sc = np.float32(1.0 / math.sqrt(DK))
    bo_eff = (np.asarray(bv, np.float32) @ Wo + np.asarray(bo, np.float32)).reshape(
        1, D
    )

    idxs, nv = [], []
    for i in range(B):
        ix = np.nonzero(np.asarray(mask[i, 0]) != 0)[0]
        idxs.append(ix)
        nv.append(len(ix))
    nkc = min(NCH, max(1, -(-max(nv) // P)))
    SK = nkc * P

    bf16 = ml_dtypes.bfloat16
    wqb = _band(Wq * sc, NCH).astype(bf16)
    wkb = _band(Wk, NCH).astype(bf16)
    wv_b = np.ascontiguousarray(Wv).astype(bf16)
    wo_b = np.ascontiguousarray(Wo).astype(bf16)
    bq2 = np.ascontiguousarray((np.asarray(bq, np.float32) * sc).reshape(NCH, P).T)
    bk2 = np.ascontiguousarray(np.asarray(bk, np.float32).reshape(NCH, P).T)

    in_maps = []
    for i in range(B):
        ix = idxs[i]
        pad = SK - len(ix)
        ixp = np.concatenate([ix, np.zeros(pad, dtype=ix.dtype)])
        mb = np.full(SK, 0.0, dtype=np.float32)
        if pad:
            mb[len(ix) :] = NEGB
        kTg = np.ascontiguousarray(key[i][ixp].astype(bf16).T)
        vT = value[i][ixp].astype(bf16).T  # [D, SK]
        vgb = np.ascontiguousarray(_band(vT, nkc))
        in_maps.append(
            {
                "qT": np.ascontiguousarray(query[i].astype(bf16).T),
                "kTg": kTg,
                "vgb": vgb,
                "wqb": wqb,
                "wkb": wkb,
                "wv": wv_b,
                "wo": wo_b,
                "bq": bq2,
                "bk": bk2,
                "msk": np.ascontiguousarray(mb.reshape(nkc, P).T),
                "bo": bo_eff,
            }
        )
    return nkc, in_maps


def kernel(query, key, value, mask, Wq, bq, Wk, bk, Wv, bv, Wo, bo):
    nkc, in_maps = _make_in_maps(
        query, key, value, mask, Wq, bq, Wk, bk, Wv, bv, Wo, bo
    )
    if nkc not in _NC_CACHE:
        _NC_CACHE[nkc] = build_nc(nkc)
    nc = _NC_CACHE[nkc]
    res = run_bass_kernel_spmd(nc, in_maps, list(range(B)))
    return np.stack([res.results[i]["out"] for i in range(B)], axis=0).astype(
        np.float32
    )
